# revision 10
# baseline (speedup 1.0000x reference)
"""Trainium2 Bass kernel for nn_CL_spatial_global (contrastive spatial loss).

Strategy: shard the spatial axis N=H*W=1024 across 8 cores (128 columns
each); all of B stays local per core, so every normalization and the
column norms of g are local.  Only the per-core partial column-sums
S = sum_b h[b] (and the column norm-sums q) cross cores, via one small
bf16 AllGather.  The [C,(b,n)] "transposed" layout falls out of the
projection matmuls naturally, so both operands of the big [N,C]@[C,N]
logit matmuls are already in the right layout with no transposes.

Per (batch, side) the logits against Gcat=[G1|G2] are computed as one
bf16 matmul group into PSUM, then a single fused ACT instruction does
exp(scale[row]*x) with a per-partition scale AP (1/(tau*sqrt(B)*||h||))
and accum_out producing the row sum (= intra_sum + inter_sum) directly.
"""

import numpy as np
import ml_dtypes

from concourse import bass, bacc, tile, mybir
from concourse.bass_utils import run_bass_kernel_spmd

F32 = mybir.dt.float32
BF16 = mybir.dt.bfloat16
ALU = mybir.AluOpType
ACTF = mybir.ActivationFunctionType

B, C, H, W = 16, 256, 32, 32
N = H * W            # 1024
P = 64               # proj hidden
TAU = 0.4
NCORES = 8
NL = N // NCORES     # 128 spatial columns per core
KC = C // 128        # 2 partition chunks of C
SCALE = 1.0 / (TAU * np.sqrt(float(B)))   # logit scale 1/(tau*sqrt(B))


def _rsqrt_newton(nc, out, x, tmp):
    """out = 1/sqrt(x) on DVE only (no ACT Sqrt): quake seed + 3 Newton iters.

    x, out, tmp: f32 SBUF APs of identical shape. x must be > 0.
    """
    i32 = mybir.dt.int32
    # seed: yi = 0x5f3759df - (xi >> 1)
    nc.vector.tensor_scalar(tmp.bitcast(i32), x.bitcast(i32), 1, None,
                            ALU.arith_shift_right)
    nc.vector.tensor_scalar(out.bitcast(i32), tmp.bitcast(i32), -1, 0x5F3759DF,
                            ALU.mult, ALU.add)
    for _ in range(3):
        # y = y * (1.5 - 0.5 * x * y^2)
        nc.vector.tensor_tensor(tmp, out, out, ALU.mult)
        nc.vector.tensor_tensor(tmp, tmp, x, ALU.mult)
        nc.vector.tensor_scalar(tmp, tmp, -0.5, 1.5, ALU.mult, ALU.add)
        nc.vector.tensor_tensor(out, out, tmp, ALU.mult)


def _build():
    nc = bacc.Bacc("TRN2", target_bir_lowering=False, debug=False,
                   num_devices=NCORES)

    # ---- kernel I/O (per core; host pre-shards / pre-permutes) ----
    z1s = nc.dram_tensor("z1s", [KC, 128, B, NL], F32, kind="ExternalInput")
    z2s = nc.dram_tensor("z2s", [KC, 128, B, NL], F32, kind="ExternalInput")
    w1t = nc.dram_tensor("w1t", [KC, 128, P], F32, kind="ExternalInput")
    b1c = nc.dram_tensor("b1c", [P, 1], F32, kind="ExternalInput")
    b1p1 = nc.dram_tensor("b1p1", [P, 1], F32, kind="ExternalInput")
    w2b = nc.dram_tensor("w2b", [KC, P + 1, 128], BF16, kind="ExternalInput")
    id16 = nc.dram_tensor("id16", [2 * NCORES, 2 * NCORES], F32,
                          kind="ExternalInput")
    out_d = nc.dram_tensor("out", [1, 1], F32, kind="ExternalOutput")

    BN = B * NL  # 2048 free elements per (kc) plane

    with tile.TileContext(nc) as tc:
        with (
            tc.tile_pool(name="const", bufs=1) as cpool,
            tc.tile_pool(name="big", bufs=1) as big,
            tc.tile_pool(name="small", bufs=1) as small,
            tc.tile_pool(name="dram", bufs=1, space="DRAM") as dram,
        ):
            # ---- constants ----
            w1s = cpool.tile([128, KC, P], F32)
            nc.sync.dma_start(w1s[:], w1t.ap().rearrange("kc c p -> c kc p"))
            w2s = cpool.tile([P + 1, KC, 128], BF16)
            nc.sync.dma_start(w2s[:], w2b.ap().rearrange("kc r m -> r kc m"))
            b1_t = cpool.tile([P, 1], F32)
            nc.sync.dma_start(b1_t[:], b1c.ap())
            b1p1_t = cpool.tile([P, 1], F32)
            nc.sync.dma_start(b1p1_t[:], b1p1.ap())
            ones_bf = cpool.tile([128, 1], BF16)
            nc.gpsimd.memset(ones_bf[:], 1.0)
            ones_f = cpool.tile([128, 1], F32)
            nc.gpsimd.memset(ones_f[:], 1.0)
            ones16 = cpool.tile([2 * NCORES, 128], F32)
            nc.gpsimd.memset(ones16[:], 1.0)
            id16_t = cpool.tile([2 * NCORES, 2 * NCORES], F32)
            nc.sync.dma_start(id16_t[:], id16.ap())

            # ---- big working tiles ----
            Z = [big.tile([128, KC, B, NL], F32, tag=f"z{i}", name=f"Z{i}")
                 for i in range(2)]
            nc.sync.dma_start(Z[0][:], z1s.ap().rearrange("kc c b n -> c kc b n"))
            nc.sync.dma_start(Z[1][:], z2s.ap().rearrange("kc c b n -> c kc b n"))
            HT = [big.tile([128, KC, B, NL], BF16, tag=f"ht{i}", name=f"HT{i}")
                  for i in range(2)]
            SSB = big.tile([128, 2, KC, NL], BF16, tag="ssb")

            nsq_t = [small.tile([128, B], F32, tag=f"nsq{i}", name=f"nsq{i}")
                     for i in range(2)]
            q_bf = [small.tile([128, 1], BF16, tag=f"qbf{i}", name=f"qbf{i}")
                    for i in range(2)]
            d12_t = small.tile([128, B], F32, tag="d12")

            with (
                tc.tile_pool(name="pm", bufs=1, space="PSUM") as pm,
                tc.tile_pool(name="ph", bufs=1, space="PSUM") as ph,
            ):
                for z in range(2):
                    # ---- layer 1: mid = w1 @ z + b1 (f32), M=64 ----
                    mid_ps = pm.tile([P, BN], F32, tag="mid")
                    for kc in range(KC):
                        for fs in range(4):
                            nc.tensor.matmul(
                                mid_ps[:, fs * 512:(fs + 1) * 512],
                                w1s[:, kc, :],
                                Z[z][:, kc, 4 * fs:4 * fs + 4, :],
                                start=(kc == 0), stop=(kc == KC - 1),
                            )
                    # ---- ELU(+1): midp = max(t+1, exp(min(t,0))), t = mid+b1
                    u_t = big.tile([P, BN], F32, tag="u")
                    nc.vector.tensor_scalar(u_t[:], mid_ps[:], b1_t[:], 0.0,
                                            ALU.add, ALU.min)
                    e_t = big.tile([P, BN], F32, tag="e")
                    nc.scalar.activation(e_t[:], u_t[:], ACTF.Exp)
                    midp = big.tile([P + 1, B, NL], BF16, tag="midp")
                    nc.vector.scalar_tensor_tensor(
                        midp[0:P, :, :].rearrange("p b n -> p (b n)"),
                        mid_ps[:], b1p1_t[:], e_t[:], ALU.add, ALU.max)
                    nc.gpsimd.memset(midp[P:P + 1, :, :], 1.0)

                    # ---- layer 2 (bf16): ht[kc] = w2' @ midp  [128, BN] ----
                    nsq_ps = None
                    sq_t = big.tile([128, BN], BF16, tag="sq")
                    sq2_t = big.tile([128, BN], BF16, tag="sq2")
                    for kc in range(KC):
                        ht_ps = ph.tile([128, BN], F32, tag="ht")
                        for fs in range(4):
                            nc.tensor.matmul(
                                ht_ps[:, fs * 512:(fs + 1) * 512],
                                w2s[:, kc, :],
                                midp[:, :, :].rearrange("p b n -> p (b n)")
                                    [:, fs * 512:(fs + 1) * 512],
                                start=True, stop=True,
                            )
                        htv = HT[z][:, kc, :, :].rearrange("c b n -> c (b n)")
                        if kc == 0:
                            nc.scalar.copy(htv, ht_ps[:])
                        else:
                            nc.vector.tensor_copy(htv, ht_ps[:])
                        # squares for column norms
                        sq = sq_t if kc == 0 else sq2_t
                        nc.vector.tensor_tensor(sq[:], htv, htv, ALU.mult)
                        # partial column sums S via halving tree over b
                        tr = big.tile([128, 8, NL], BF16, tag="tree")
                        nc.vector.tensor_tensor(
                            tr[:], HT[z][:, kc, 0:8, :], HT[z][:, kc, 8:16, :],
                            ALU.add)
                        nc.vector.tensor_tensor(
                            tr[:, 0:4, :], tr[:, 0:4, :], tr[:, 4:8, :], ALU.add)
                        nc.vector.tensor_tensor(
                            tr[:, 0:2, :], tr[:, 0:2, :], tr[:, 2:4, :], ALU.add)
                        nc.vector.tensor_tensor(
                            SSB[:, z, kc, :], tr[:, 0, :], tr[:, 1, :], ALU.add)
                    # ---- nsq[(b,n)] = sum_c ht^2 : ones-matmul over partitions
                    nsq_ps = pm.tile([1, BN], F32, tag="mid")
                    for kc in range(KC):
                        sq = sq_t if kc == 0 else sq2_t
                        for fs in range(4):
                            nc.tensor.matmul(
                                nsq_ps[:, fs * 512:(fs + 1) * 512],
                                ones_bf[:],
                                sq[:, fs * 512:(fs + 1) * 512],
                                start=(kc == 0), stop=(kc == KC - 1),
                            )
                    # [1,(b,n)] -> [16,128] (contiguous) -> PE transpose
                    nsq_s = small.tile([1, BN], F32, tag="nsqs",
                                       name=f"nsqs{z}")
                    nc.vector.tensor_copy(nsq_s[:], nsq_ps[:])
                    nsq_d = dram.tile([1, BN], F32, tag="nsqd",
                                      name=f"nsqd{z}")
                    nc.sync.dma_start(nsq_d[:], nsq_s[:])
                    nsqT = small.tile([B, NL], F32, tag="nsqT",
                                      name=f"nsqT{z}")
                    nc.sync.dma_start(
                        nsqT[:],
                        nsq_d[:].rearrange("p (b n) -> (p b) n", b=B))
                    tp_ps = ph.tile([128, B], F32, tag="ht", name=f"tpn{z}")
                    nc.tensor.transpose(tp_ps[:], nsqT[:], id16_t[:])
                    nc.vector.tensor_copy(nsq_t[z][:], tp_ps[:])
                    # q[n] = sum_b nsq  (f32 -> bf16 for the AllGather)
                    qf = small.tile([128, 1], F32, tag=f"qf{z}", name=f"qf{z}")
                    nc.vector.tensor_reduce(qf[:], nsq_t[z][:],
                                            mybir.AxisListType.X, ALU.add)
                    nc.vector.tensor_copy(q_bf[z][:], qf[:])

                # ---- d12[(b,n)] = sum_c ht1*ht2 ----
                p12_t = big.tile([128, BN], BF16, tag="sq")
                p122_t = big.tile([128, BN], BF16, tag="sq2")
                d12_ps = pm.tile([1, BN], F32, tag="mid")
                for kc in range(KC):
                    p12 = p12_t if kc == 0 else p122_t
                    nc.vector.tensor_tensor(
                        p12[:],
                        HT[0][:, kc, :, :].rearrange("c b n -> c (b n)"),
                        HT[1][:, kc, :, :].rearrange("c b n -> c (b n)"),
                        ALU.mult)
                    for fs in range(4):
                        nc.tensor.matmul(
                            d12_ps[:, fs * 512:(fs + 1) * 512],
                            ones_bf[:],
                            p12[:, fs * 512:(fs + 1) * 512],
                            start=(kc == 0), stop=(kc == KC - 1),
                        )
                d12_s = small.tile([1, BN], F32, tag="d12s")
                nc.vector.tensor_copy(d12_s[:], d12_ps[:])
                d12_d = dram.tile([1, BN], F32, tag="d12d")
                nc.sync.dma_start(d12_d[:], d12_s[:])
                d12T = small.tile([B, NL], F32, tag="d12T")
                nc.sync.dma_start(
                    d12T[:],
                    d12_d[:].rearrange("p (b n) -> (p b) n", b=B))
                tp_ps2 = ph.tile([128, B], F32, tag="ht")
                nc.tensor.transpose(tp_ps2[:], d12T[:], id16_t[:])
                nc.vector.tensor_copy(d12_t[:], tp_ps2[:])

                # ---- AllGather of [S1|S2|q1|q2] (bf16) ----
                agin = dram.tile([2, 2 * 128 + 1, NL], BF16)
                agout = dram.tile([NCORES, 2, 2 * 128 + 1, NL], BF16)
                for z in range(2):
                    for kc in range(KC):
                        nc.sync.dma_start(
                            agin[z, kc * 128:(kc + 1) * 128, :],
                            SSB[:, z, kc, :])
                    nc.sync.dma_start(agin[z, 256, :], q_bf[z][:, 0])
                nc.gpsimd.collective_compute(
                    "AllGather", ALU.bypass,
                    replica_groups=[list(range(NCORES))],
                    ins=[agin[:].opt()],
                    outs=[agout[:].opt()],
                )
                SF = [big.tile([128, KC, NCORES, NL], BF16, tag=f"sf{i}",
                               name=f"SF{i}") for i in range(2)]
                qgT_bf = small.tile([2 * NCORES, NL], BF16, tag="qgtb")
                for z in range(2):
                    for kc in range(KC):
                        nc.sync.dma_start(
                            SF[z][:, kc, :, :],
                            agout[:, z, kc * 128:(kc + 1) * 128, :]
                                .rearrange("r c n -> c r n"))
                    nc.sync.dma_start(
                        qgT_bf[z * NCORES:(z + 1) * NCORES, :],
                        agout[:, z, 256, :])

                # ---- invg = 1/sqrt(qg) on [16,128]; block-diag broadcast ----
                qgT_f = small.tile([2 * NCORES, NL], F32, tag="qgtf")
                nc.vector.tensor_copy(qgT_f[:], qgT_bf[:])
                invgT = small.tile([2 * NCORES, NL], F32, tag="invgT")
                tmp_g = small.tile([2 * NCORES, NL], F32, tag="tmpg")
                _rsqrt_newton(nc, invgT[:], qgT_f[:], tmp_g[:])
                bd = small.tile([2 * NCORES, 2 * N], F32, tag="bd")
                nc.gpsimd.memset(bd[:], 0.0)
                for k in range(2 * NCORES):
                    nc.sync.dma_start(bd[k:k + 1, k * NL:(k + 1) * NL],
                                      invgT[k:k + 1, :])
                invgb_ps = pm.tile([128, 2 * N], F32, tag="mid")
                for fs in range(4):
                    nc.tensor.matmul(
                        invgb_ps[:, fs * 512:(fs + 1) * 512],
                        ones16[:],
                        bd[:, fs * 512:(fs + 1) * 512],
                        start=True, stop=True,
                    )
                # ---- Gcat = S_full * invg (bf16): cols [G1 | G2] ----
                GC = big.tile([128, KC, 2 * N], BF16, tag="gcat")
                for z in range(2):
                    for kc in range(KC):
                        nc.vector.tensor_tensor(
                            GC[:, kc, z * N:(z + 1) * N],
                            SF[z][:, kc, :, :].rearrange("c r n -> c (r n)"),
                            invgb_ps[:, z * N:(z + 1) * N],
                            ALU.mult)

            # ---- per-(b,n) scales and diagonal terms (all [128, B]) ----
            inv_t = small.tile([128, 2 * B], F32, tag="inv")
            nsq_cat = small.tile([128, 2 * B], F32, tag="nsqcat")
            nc.vector.tensor_copy(nsq_cat[:, 0:B], nsq_t[0][:])
            nc.vector.tensor_copy(nsq_cat[:, B:2 * B], nsq_t[1][:])
            tmp_i = small.tile([128, 2 * B], F32, tag="tmpi")
            _rsqrt_newton(nc, inv_t[:], nsq_cat[:], tmp_i[:])
            invs_t = small.tile([128, 2 * B], F32, tag="invs")
            nc.vector.tensor_scalar(invs_t[:], inv_t[:], float(SCALE), None,
                                    ALU.mult)
            # intra_diag = exp(nsq * inv^2 / tau)
            intra_t = small.tile([128, 2 * B], F32, tag="intra")
            nc.vector.tensor_tensor(tmp_i[:], nsq_cat[:], inv_t[:], ALU.mult)
            nc.vector.tensor_tensor(tmp_i[:], tmp_i[:], inv_t[:], ALU.mult)
            nc.scalar.activation(intra_t[:], tmp_i[:], ACTF.Exp,
                                 scale=1.0 / TAU)
            # x12 = d12 * inv1 * inv2 / tau
            x12_t = small.tile([128, B], F32, tag="x12")
            nc.vector.tensor_tensor(x12_t[:], d12_t[:], inv_t[:, 0:B], ALU.mult)
            nc.vector.tensor_tensor(x12_t[:], x12_t[:], inv_t[:, B:2 * B],
                                    ALU.mult)

            # ---- main loop: logits + fused exp/rowsum ----
            rows_t = small.tile([128, 2 * B], F32, tag="rows")
            exps = big.tile([128, 2 * N], BF16, tag="exps")
            with tc.tile_pool(name="pb", bufs=2, space="PSUM") as pb:
                for b in range(B):
                    for z in range(2):
                        ps = pb.tile([128, 2 * N], F32, tag="log")
                        for kc in range(KC):
                            lhsT = HT[z][:, kc, b, :]
                            for fs in range(4):
                                nc.tensor.matmul(
                                    ps[:, fs * 512:(fs + 1) * 512],
                                    lhsT,
                                    GC[:, kc, fs * 512:(fs + 1) * 512],
                                    start=(kc == 0), stop=(kc == KC - 1),
                                )
                        col = z * B + b
                        nc.scalar.activation(
                            exps[:], ps[:], ACTF.Exp,
                            scale=invs_t[:, col:col + 1],
                            accum_out=rows_t[:, col:col + 1])

                # ---- loss assembly ----
                d_t = small.tile([128, 2 * B], F32, tag="dt")
                nc.vector.tensor_tensor(d_t[:], rows_t[:], intra_t[:],
                                        ALU.subtract)
                dln = small.tile([128, 2 * B], F32, tag="dln")
                lsum = small.tile([128, 1], F32, tag="lsum")
                nc.scalar.activation(dln[:], d_t[:], ACTF.Ln,
                                     accum_out=lsum[:])
                xs = small.tile([128, 1], F32, tag="xs")
                nc.vector.tensor_reduce(xs[:], x12_t[:],
                                        mybir.AxisListType.X, ALU.add)
                xs2 = small.tile([128, 1], F32, tag="xs2")
                nc.vector.tensor_scalar(xs2[:], xs[:], 1.0 / TAU, None,
                                        ALU.mult)
                total = small.tile([128, 1], F32, tag="total")
                nc.vector.scalar_tensor_tensor(total[:], lsum[:], 0.5, xs2[:],
                                               ALU.mult, ALU.subtract)
                fin_ps = pb.tile([1, 1], F32, tag="log")
                nc.tensor.matmul(fin_ps[:], total[:], ones_f[:],
                                 start=True, stop=True)
                out_sb = small.tile([1, 1], F32, tag="outsb")
                nc.vector.tensor_copy(out_sb[:], fin_ps[:])
                nc.sync.dma_start(out_d[:], out_sb[:])

    nc.compile()
    return nc


def _prep_inputs(z1, z2, fc1_w, fc1_b, fc2_w, fc2_b):
    """Host-side sharding/permutation into per-core input maps."""
    bf = ml_dtypes.bfloat16
    w1t = np.ascontiguousarray(
        fc1_w.T.reshape(KC, 128, P).astype(np.float32))
    b1c = np.ascontiguousarray(fc1_b.reshape(P, 1).astype(np.float32))
    b1p1 = np.ascontiguousarray((fc1_b + 1.0).reshape(P, 1).astype(np.float32))
    # layer-2 weights with "-1" fold: b2' = b2 - sum_p w2[:, p]
    b2p = (fc2_b - fc2_w.sum(axis=1)).astype(np.float32)
    w2T = fc2_w.T.astype(np.float32)          # [P, C]
    w2b = np.zeros((KC, P + 1, 128), np.float32)
    for kc in range(KC):
        w2b[kc, 0:P, :] = w2T[:, kc * 128:(kc + 1) * 128]
        w2b[kc, P, :] = b2p[kc * 128:(kc + 1) * 128]
    w2b = w2b.astype(bf)

    z1r = z1.reshape(B, KC, 128, N).transpose(1, 2, 0, 3)  # [kc, c, b, n]
    z2r = z2.reshape(B, KC, 128, N).transpose(1, 2, 0, 3)
    in_maps = []
    for r in range(NCORES):
        sl = slice(r * NL, (r + 1) * NL)
        in_maps.append({
            "z1s": np.ascontiguousarray(z1r[:, :, :, sl], dtype=np.float32),
            "z2s": np.ascontiguousarray(z2r[:, :, :, sl], dtype=np.float32),
            "w1t": w1t, "b1c": b1c, "b1p1": b1p1, "w2b": w2b,
            "id16": np.eye(2 * NCORES, dtype=np.float32),
        })
    return in_maps


_CACHED_NC = None


def _get_nc():
    global _CACHED_NC
    if _CACHED_NC is None:
        _CACHED_NC = _build()
    return _CACHED_NC


def kernel(z1, z2, fc1_w, fc1_b, fc2_w, fc2_b):
    nc = _get_nc()
    in_maps = _prep_inputs(z1, z2, fc1_w, fc1_b, fc2_w, fc2_b)
    res = run_bass_kernel_spmd(nc, in_maps, list(range(NCORES)))
    total = sum(float(res.results[r]["out"][0, 0]) for r in range(NCORES))
    return np.float32(total / (B * N))
